# revision 18
# baseline (speedup 1.0000x reference)
"""CapsulePooling2D kernel for Trainium2, 8-core batch-data-parallel.

Full input x: (32, 64, 64, 256) fp32 -> output (32, 32, 32, 256) fp32.
Each NeuronCore handles 4 samples.

Math (per independent 2x2 spatial block of 4 pixels p0..p3, 256 channels):
  G[a,b] = x_a . x_b                      (4x4 Gram, from fp16-rounded x)
  step1: d1 = G @ 1 ; sigma1 = sum(G);  s1 = d1 / (4*(1+EPS) + sqrt(sigma1))
  step2: e2 = exp((s1 - max s1)/2); u = G @ e2
         s2 = u / ((sum e2 + 4EPS)*(1+EPS) + sqrt(e2^T G e2)); score = s1+s2
  step3: e3 = exp((score - max score)/2); w = e3 / (sum e3 + 4EPS)
         out = sum_k w_k * x_k
(The reference's step-3 score update is dead code for the output.)

Implementation notes (v7):
  Layout: tile = 128 blocks on partitions (4 block-rows x 32 block-cols),
  free dim = (tile 32, pixel 4, channel 256); x cast to fp16 on the host
  (HWDGE input DMAs on Sync; GpSimd and its SWDGE drains unused).  Sample 0 is loaded with per-(tile, row-parity) DMAs so
  the first Gram chunk starts ~8us in; samples 1-3 use batched
  per-(sample, block-row, parity) DMAs to keep GpSimd descriptor
  generation cheap.  Gram runs in chunks of 4/8 tiles overlapped with
  the input DMA: ScalarE squares the 4 diagonal pixels (batched ACT per
  chunk, double-buffered pool so it runs a chunk ahead of VectorE),
  VectorE forms the 6 off-diagonal products as big fp16 tensor_tensor
  ops and reduces over channels with an fp16 halving tree (each level
  slices in0/in1 halves so op cost equals output size) down to width 8,
  finished by segmented tensor_reduce into the fp32 G layout.  The
  per-block scalar algebra is batched once across all 32 tiles.  The
  weighted output sum runs per sample-quarter: per-pixel-slot products
  (w broadcast via width-4 fp16 replica + stride-0 AP dim) into k-major
  slabs, 3 fp16 adds, then immediate per-(sample, block-row) output
  DMAs split across the Sync and ScalarE HWDGE queues so only the last
  quarter's DMA is kernel tail.  Output is fp16, upcast on the host.
  All activations are forced onto the natural_log_exp_and_others table
  set (sqrt via exp(0.5*ln)) so ACT loads tables exactly once.
"""

import sys

if "/opt/trn_rl_repo" not in sys.path:
    sys.path.insert(0, "/opt/trn_rl_repo")

import numpy as np

N_CORES = 8
B_FULL, H, W, C = 32, 64, 64, 256
B = B_FULL // N_CORES          # 4 samples per core
HO, WO = H // 2, W // 2
ROWS_PER_TILE = 8              # image rows per tile -> 4 block-rows x 32 blocks
TILES_PER_SAMPLE = H // ROWS_PER_TILE  # 8
NT = B * TILES_PER_SAMPLE      # 32 tiles per core
NB = NT
TC = TILES_PER_SAMPLE          # max chunk = 1 sample = 8 tiles
EPS = 1e-7
CHUNKS = [(0, 2), (2, 4), (4, 8), (8, 16), (16, 24), (24, 32)]

OFF_PAIRS = [(0, 1), (0, 2), (0, 3), (1, 2), (1, 3), (2, 3)]

_cache = {}


def _build_nc():
    import concourse.bacc as bacc
    import concourse.tile as tile
    import concourse.mybir as mybir

    fp32 = mybir.dt.float32
    fp16 = mybir.dt.float16
    F = mybir.ActivationFunctionType
    OP = mybir.AluOpType
    AX = mybir.AxisListType

    import concourse.hw_specs as hw_specs
    if not hasattr(hw_specs, "_orig_get_activation_tables"):
        hw_specs._orig_get_activation_tables = hw_specs.get_activation_tables

        def _one_set(arch):
            tabs = hw_specs._orig_get_activation_tables(arch)
            if "natural_log_exp_and_others" not in tabs:
                return tabs
            return {
                k: (v if k == "natural_log_exp_and_others" else set())
                for k, v in tabs.items()
            }

        bacc.get_activation_tables = _one_set

    nc = bacc.Bacc("TRN2", num_devices=N_CORES)
    x = nc.dram_tensor("x", [B, H, W, C], fp16, kind="ExternalInput").ap()
    out = nc.dram_tensor("out", [B, HO, WO, C], fp16, kind="ExternalOutput").ap()

    with tile.TileContext(nc) as tc:
        with (
            tc.tile_pool(name="xp", bufs=1) as xp,
            tc.tile_pool(name="gp", bufs=1) as gp,
            tc.tile_pool(name="sp", bufs=2) as sp,
        ):
            X = xp.tile([128, NT * 4 * C], fp16, tag="X", name="X")
            Xv = X[:].rearrange("p (t k c) -> p t k c", t=NT, k=4)
            G_all = gp.tile([128, NT * 16], fp32, tag="G", name="G_all")
            Gv = G_all[:].rearrange("p (t a b) -> p t a b", t=NT, a=4)
            # G viewed slot-major: [p, slot 16, tile 32]
            Gst = G_all[:].rearrange("p (t s) -> p s t", t=NT)

            # ---- input DMA (SWDGE fp32->fp16 cast on GpSimd) ----
            # sample 0: per (tile, di) for a fast first chunk;
            # samples 1-3: per (ii, di) batched over their 8 tiles.
            for t in range(TC):
                src = x[0, ROWS_PER_TILE * t : ROWS_PER_TILE * (t + 1), :, :]
                src = src.rearrange(
                    "(ii di) (j dj) c -> ii di j (dj c)", di=2, dj=2
                )
                for di in range(2):
                    nc.sync.dma_start(
                        out=Xv[:, t, 2 * di : 2 * di + 2, :],
                        in_=src[:, di, :, :],
                    )
            for s in range(1, B):
                srcv = x[s].rearrange(
                    "(ti ii di) (j dj) c -> ii di ti j (dj c)",
                    ti=TC, di=2, dj=2,
                )
                dstv = Xv[:, s * TC : (s + 1) * TC, :, :].rearrange(
                    "p t (di dj) c -> p di t (dj c)", di=2
                )
                for ii in range(4):
                    for di in range(2):
                        nc.sync.dma_start(
                            out=dstv[32 * ii : 32 * (ii + 1), di, :, :],
                            in_=srcv[ii, di].rearrange("t j r -> j t r"),
                        )

            # warm the ACT table set while DMA streams
            warm = sp.tile([128, 1], fp32, tag="warm", name="warm")
            nc.vector.memset(warm[:], 0.0)
            nc.scalar.activation(warm[:], warm[:], F.Exp)

            # ---- Gram, per chunk ----
            if True:
                _p6cm = tc.tile_pool(name="p6", bufs=1)
                _p4cm = tc.tile_pool(name="p4", bufs=2)
                p6 = _p6cm.__enter__()
                p4 = _p4cm.__enter__()
                A6 = p6.tile([128, 6 * TC * 256], fp16, tag="A6", name="A6")
                B6 = p6.tile([128, 6 * TC * 128], fp16, tag="B6", name="B6")
                B4 = p6.tile([128, 4 * TC * 128], fp16, tag="B4", name="B4")

                def reduce_groups(srcv, groups, tlo, tcn, width):
                    for (r0, r1, s0) in groups:
                        nc.vector.tensor_reduce(
                            Gst[:, s0 : s0 + (r1 - r0), tlo : tlo + tcn],
                            srcv[:, r0:r1, :, 0:width],
                            axis=AX.X,
                            op=OP.add,
                        )

                def tree(Abuf, Bbuf, npair, groups, tlo, tcn):
                    Av = Abuf[:].rearrange(
                        "p (r t c) -> p r t c", r=npair, t=TC
                    )[:, :, 0:tcn, :]
                    Bv = Bbuf[:].rearrange(
                        "p (r t c) -> p r t c", r=npair, t=TC
                    )[:, :, 0:tcn, :]
                    w = 256
                    src, dst = Av, Bv
                    while w > 8:
                        h = w // 2
                        nc.vector.tensor_tensor(
                            out=dst[:, :, :, 0:h],
                            in0=src[:, :, :, 0:h],
                            in1=src[:, :, :, h:w],
                            op=OP.add,
                        )
                        src, dst = dst, src
                        w = h
                    reduce_groups(src, groups, tlo, tcn, 8)

                OFF_GROUPS = [(0, 3, 1), (3, 5, 6), (5, 6, 11)]
                DIAG_GROUPS = [(0, 1, 0), (1, 2, 5), (2, 3, 10), (3, 4, 15)]

                for (tlo, thi) in CHUNKS:
                    tcn = thi - tlo
                    ts = slice(tlo, thi)
                    A4 = p4.tile(
                        [128, 4 * TC * 256], fp16, tag="A4", name="A4"
                    )
                    A4v = A4[:].rearrange(
                        "p (r t c) -> p r t c", r=4, t=TC
                    )[:, :, 0:tcn, :]
                    for a in range(4):
                        nc.scalar.activation(
                            A4v[:, a, :, :], Xv[:, ts, a, :], F.Square
                        )
                    A6v = A6[:].rearrange(
                        "p (r t c) -> p r t c", r=6, t=TC
                    )[:, :, 0:tcn, :]
                    for r, (a, b) in enumerate(OFF_PAIRS):
                        nc.vector.tensor_tensor(
                            out=A6v[:, r, :, :],
                            in0=Xv[:, ts, a, :],
                            in1=Xv[:, ts, b, :],
                            op=OP.mult,
                        )
                    tree(A6, B6, 6, OFF_GROUPS, tlo, tcn)
                    tree(A4, B4, 4, DIAG_GROUPS, tlo, tcn)

            # ---- scalar algebra, batched across all tiles ----
            def t4(name):
                return sp.tile([128, NB * 4], fp32, tag=name, name=name)

            def t1(name):
                return sp.tile([128, NB], fp32, tag=name, name=name)

            def sqrt_ln_exp(dst, src, tmp):
                nc.scalar.activation(tmp[:], src[:], F.Ln)
                nc.scalar.activation(dst[:], tmp[:], F.Exp, scale=0.5)

            nc.vector.tensor_copy(Gv[:, :, 1:4, 0], Gv[:, :, 0, 1:4])
            nc.vector.tensor_copy(Gv[:, :, 2:4, 1], Gv[:, :, 1, 2:4])
            nc.vector.tensor_copy(Gv[:, :, 3, 2:3], Gv[:, :, 2, 3:4])

            d1 = t4("d1")
            d1v = d1[:].rearrange("p (t k) -> p t k", t=NB)
            nc.vector.tensor_reduce(d1v, Gv, axis=AX.X, op=OP.add)
            sig1 = t1("sig1")
            nc.vector.tensor_reduce(sig1[:], d1v, axis=AX.X, op=OP.add)
            sA = t1("sA")
            tmp1 = t1("tmp1")
            sqrt_ln_exp(sA, sig1, tmp1)
            den1 = t1("den1")
            nc.vector.tensor_scalar_add(den1[:], sA[:], 4.0 * (1.0 + EPS))
            r1 = t1("r1")
            nc.vector.reciprocal(r1[:], den1[:])
            score = t4("score")
            scv = score[:].rearrange("p (t k) -> p t k", t=NB)
            nc.vector.tensor_tensor(
                out=scv,
                in0=d1v,
                in1=r1[:].unsqueeze(2).broadcast_to([128, NB, 4]),
                op=OP.mult,
            )

            def softmax_weights(sc_v, ename):
                e = t4("e" + ename)
                ev = e[:].rearrange("p (t k) -> p t k", t=NB)
                sc_flat = sc_v.rearrange("p t k -> p (t k)")
                nc.scalar.activation(e[:], sc_flat, F.Exp, scale=0.5)
                se = t1("se" + ename)
                nc.vector.tensor_reduce(se[:], ev, axis=AX.X, op=OP.add)
                return ev, se

            e2v, se2 = softmax_weights(scv, "2")
            P4s = sp.tile([128, NB * 16], fp32, tag="P4s", name="P4s")
            P4v = P4s[:].rearrange("p (t a b) -> p t a b", t=NB, a=4)
            nc.vector.tensor_tensor(
                out=P4v,
                in0=Gv,
                in1=e2v.unsqueeze(2).broadcast_to([128, NB, 4, 4]),
                op=OP.mult,
            )
            u = t4("u")
            uv = u[:].rearrange("p (t k) -> p t k", t=NB)
            nc.vector.tensor_reduce(uv, P4v, axis=AX.X, op=OP.add)
            eu = t4("eu")
            euv = eu[:].rearrange("p (t k) -> p t k", t=NB)
            nc.vector.tensor_tensor(out=euv, in0=e2v, in1=uv, op=OP.mult)
            btb = t1("btb")
            nc.vector.tensor_reduce(btb[:], euv, axis=AX.X, op=OP.add)
            sB = t1("sB")
            tmp2 = t1("tmp2")
            sqrt_ln_exp(sB, btb, tmp2)
            DEN = t1("DEN")
            nc.vector.tensor_scalar(
                out=DEN[:],
                in0=se2[:],
                scalar1=4.0 * EPS,
                scalar2=1.0 + EPS,
                op0=OP.add,
                op1=OP.mult,
            )
            den2 = t1("den2")
            nc.vector.tensor_tensor(out=den2[:], in0=DEN[:], in1=sB[:], op=OP.add)
            r2 = t1("r2")
            nc.vector.reciprocal(r2[:], den2[:])
            s2t = t4("s2t")
            s2tv = s2t[:].rearrange("p (t k) -> p t k", t=NB)
            nc.vector.tensor_tensor(
                out=s2tv,
                in0=uv,
                in1=r2[:].unsqueeze(2).broadcast_to([128, NB, 4]),
                op=OP.mult,
            )
            score2 = t4("score2")
            sc2v = score2[:].rearrange("p (t k) -> p t k", t=NB)
            nc.vector.tensor_tensor(out=sc2v, in0=scv, in1=s2tv, op=OP.add)

            e3v, se3 = softmax_weights(sc2v, "3")
            den3 = t1("den3")
            nc.vector.tensor_scalar_add(den3[:], se3[:], 4.0 * EPS)
            q3 = t1("q3")
            nc.vector.reciprocal(q3[:], den3[:])
            W4 = sp.tile([128, NB * 4 * 4], fp16, tag="W4", name="W4")
            W4v = W4[:].rearrange("p (t k w) -> p t k w", t=NB, k=4)
            nc.vector.tensor_tensor(
                out=W4v[:, :, :, 0],
                in0=e3v,
                in1=q3[:].unsqueeze(2).broadcast_to([128, NB, 4]),
                op=OP.mult,
            )
            nc.vector.tensor_copy(W4v[:, :, :, 1:2], W4v[:, :, :, 0:1])
            nc.vector.tensor_copy(W4v[:, :, :, 2:4], W4v[:, :, :, 0:2])

            # ---- weighted output sum, per sample quarter ----
            if True:
                _wpcm = tc.tile_pool(name="wp", bufs=1)
                _wocm = tc.tile_pool(name="wo", bufs=2)
                wp = _wpcm.__enter__()
                wo = _wocm.__enter__()
                for q in range(4):
                    ts = slice(q * TC, (q + 1) * TC)
                    Pk = [
                        wp.tile([128, TC * C], fp16, tag=f"P{k}", name=f"P{k}")
                        for k in range(4)
                    ]
                    for k in range(4):
                        Pg = Pk[k][:].rearrange(
                            "p (t g w) -> p t g w", t=TC, w=4
                        )
                        Xg = Xv[:, ts, k, :].rearrange(
                            "p t (g w) -> p t g w", w=4
                        )
                        Wb = W4v[:, ts, k, :].unsqueeze(2).broadcast_to(
                            [128, TC, C // 4, 4]
                        )
                        nc.vector.tensor_tensor(
                            out=Pg, in0=Xg, in1=Wb, op=OP.mult
                        )
                    A01 = wp.tile([128, TC * C], fp16, tag="A01", name="A01")
                    A23 = wp.tile([128, TC * C], fp16, tag="A23", name="A23")
                    O = wo.tile([128, TC * C], fp16, tag="O", name="O")
                    nc.vector.tensor_tensor(
                        out=A01[:], in0=Pk[0][:], in1=Pk[1][:], op=OP.add
                    )
                    nc.vector.tensor_tensor(
                        out=A23[:], in0=Pk[2][:], in1=Pk[3][:], op=OP.add
                    )
                    nc.vector.tensor_tensor(
                        out=O[:], in0=A01[:], in1=A23[:], op=OP.add
                    )
                    Ov = O[:].rearrange("p (t c) -> p t c", t=TC)
                    dstv = out[q].rearrange("(ti ii) j c -> ii j ti c", ii=4)
                    for ii in range(4):
                        eng = nc.sync if (ii % 2 == 0) else nc.scalar
                        eng.dma_start(
                            out=dstv[ii],
                            in_=Ov[32 * ii : 32 * (ii + 1), :, :],
                        )

            _wocm.__exit__(None, None, None)
            _wpcm.__exit__(None, None, None)
            _p4cm.__exit__(None, None, None)
            _p6cm.__exit__(None, None, None)

    nc.compile()
    return nc


def _get_nc():
    if "nc" not in _cache:
        _cache["nc"] = _build_nc()
    return _cache["nc"]


def run_sharded(x, trace=False, **kw):
    from concourse.bass_utils import run_bass_kernel_spmd

    nc = _get_nc()
    x = np.ascontiguousarray(np.asarray(x)).astype(np.float16)
    in_maps = [{"x": x[i * B : (i + 1) * B]} for i in range(N_CORES)]
    res = run_bass_kernel_spmd(
        nc, in_maps, core_ids=list(range(N_CORES)), trace=trace, **kw
    )
    full = np.concatenate(
        [res.results[i]["out"] for i in range(N_CORES)], axis=0
    ).astype(np.float32)
    return full, res


def kernel(x):
    full, _ = run_sharded(x)
    return full


# revision 20
# speedup vs baseline: 1.0581x; 1.0581x over previous
"""CapsulePooling2D kernel for Trainium2, 8-core batch-data-parallel.

Full input x: (32, 64, 64, 256) fp32 -> output (32, 32, 32, 256) fp32.
Each NeuronCore handles 4 samples.

Math (per independent 2x2 spatial block of 4 pixels p0..p3, 256 channels):
  G[a,b] = x_a . x_b                      (4x4 Gram, from fp16-rounded x)
  step1: d1 = G @ 1 ; sigma1 = sum(G);  s1 = d1 / (4*(1+EPS) + sqrt(sigma1))
  step2: e2 = exp((s1 - max s1)/2); u = G @ e2
         s2 = u / ((sum e2 + 4EPS)*(1+EPS) + sqrt(e2^T G e2)); score = s1+s2
  step3: e3 = exp((score - max score)/2); w = e3 / (sum e3 + 4EPS)
         out = sum_k w_k * x_k
(The reference's step-3 score update is dead code for the output.)

Implementation notes (v8):
  Layout: tile = 128 blocks on partitions (4 block-rows x 32 block-cols),
  free dim = (tile 32, pixel 4, channel 256); x stored fp16 (SWDGE cast
  during DMA).  Sample 0 is loaded with per-(tile, row-parity) DMAs so
  the first Gram chunk starts ~8us in; samples 1-3 use batched
  per-(sample, block-row, parity) DMAs to keep GpSimd descriptor
  generation cheap.  Gram runs in chunks of 4/8 tiles overlapped with
  the input DMA: ScalarE squares the 4 diagonal pixels (batched ACT per
  chunk, double-buffered pool so it runs a chunk ahead of VectorE),
  VectorE forms the 6 off-diagonal products as big fp16 tensor_tensor
  ops and reduces over channels with an fp16 halving tree (each level
  slices in0/in1 halves so op cost equals output size) down to width 8,
  finished by segmented tensor_reduce into the fp32 G layout.  The
  per-block scalar algebra is batched once across all 32 tiles.  The
  weighted output sum runs per sample-quarter: per-pixel-slot products
  (w broadcast via width-4 fp16 replica + stride-0 AP dim) into k-major
  slabs, 3 fp16 adds, then immediate per-(sample, block-row) output
  DMAs split across the Sync and ScalarE HWDGE queues so only the last
  quarter's DMA is kernel tail.  Output is fp16, upcast on the host.
  All activations are forced onto the natural_log_exp_and_others table
  set (sqrt via exp(0.5*ln)) so ACT loads tables exactly once.
"""

import sys

if "/opt/trn_rl_repo" not in sys.path:
    sys.path.insert(0, "/opt/trn_rl_repo")

import numpy as np

N_CORES = 8
B_FULL, H, W, C = 32, 64, 64, 256
B = B_FULL // N_CORES          # 4 samples per core
HO, WO = H // 2, W // 2
ROWS_PER_TILE = 8              # image rows per tile -> 4 block-rows x 32 blocks
TILES_PER_SAMPLE = H // ROWS_PER_TILE  # 8
NT = B * TILES_PER_SAMPLE      # 32 tiles per core
NB = NT
TC = TILES_PER_SAMPLE          # max chunk = 1 sample = 8 tiles
EPS = 1e-7
CHUNKS = [(0, 2), (2, 4), (4, 8), (8, 16), (16, 24), (24, 32)]

OFF_PAIRS = [(0, 1), (0, 2), (0, 3), (1, 2), (1, 3), (2, 3)]

_cache = {}


def _build_nc():
    import concourse.bacc as bacc
    import concourse.tile as tile
    import concourse.mybir as mybir

    fp32 = mybir.dt.float32
    fp16 = mybir.dt.float16
    F = mybir.ActivationFunctionType
    OP = mybir.AluOpType
    AX = mybir.AxisListType

    import concourse.hw_specs as hw_specs
    if not hasattr(hw_specs, "_orig_get_activation_tables"):
        hw_specs._orig_get_activation_tables = hw_specs.get_activation_tables

        def _one_set(arch):
            tabs = hw_specs._orig_get_activation_tables(arch)
            if "natural_log_exp_and_others" not in tabs:
                return tabs
            return {
                k: (v if k == "natural_log_exp_and_others" else set())
                for k, v in tabs.items()
            }

        bacc.get_activation_tables = _one_set

    nc = bacc.Bacc("TRN2", num_devices=N_CORES)
    x = nc.dram_tensor("x", [B, H, W, C], fp32, kind="ExternalInput").ap()
    out = nc.dram_tensor("out", [B, HO, WO, C], fp16, kind="ExternalOutput").ap()

    with tile.TileContext(nc) as tc:
        with (
            tc.tile_pool(name="xp", bufs=1) as xp,
            tc.tile_pool(name="gp", bufs=1) as gp,
            tc.tile_pool(name="sp", bufs=2) as sp,
        ):
            X = xp.tile([128, NT * 4 * C], fp16, tag="X", name="X")
            Xv = X[:].rearrange("p (t k c) -> p t k c", t=NT, k=4)
            G_all = gp.tile([128, NT * 16], fp32, tag="G", name="G_all")
            Gv = G_all[:].rearrange("p (t a b) -> p t a b", t=NT, a=4)
            # G viewed slot-major: [p, slot 16, tile 32]
            Gst = G_all[:].rearrange("p (t s) -> p s t", t=NT)

            # ---- input DMA (SWDGE fp32->fp16 cast on GpSimd) ----
            # sample 0: per (tile, di) for a fast first chunk;
            # samples 1-3: per (ii, di) batched over their 8 tiles.
            for t in range(TC):
                src = x[0, ROWS_PER_TILE * t : ROWS_PER_TILE * (t + 1), :, :]
                src = src.rearrange(
                    "(ii di) (j dj) c -> ii di j (dj c)", di=2, dj=2
                )
                for di in range(2):
                    nc.gpsimd.dma_start(
                        out=Xv[:, t, 2 * di : 2 * di + 2, :],
                        in_=src[:, di, :, :],
                    )
            def issue_sample_dma(s):
                srcv = x[s].rearrange(
                    "(ti ii di) (j dj) c -> ii di ti j (dj c)",
                    ti=TC, di=2, dj=2,
                )
                dstv = Xv[:, s * TC : (s + 1) * TC, :, :].rearrange(
                    "p t (di dj) c -> p di t (dj c)", di=2
                )
                for ii in range(4):
                    for di in range(2):
                        nc.gpsimd.dma_start(
                            out=dstv[32 * ii : 32 * (ii + 1), di, :, :],
                            in_=srcv[ii, di].rearrange("t j r -> j t r"),
                        )

            # warm the ACT table set while DMA streams
            warm = sp.tile([128, 1], fp32, tag="warm", name="warm")
            nc.vector.memset(warm[:], 0.0)
            nc.scalar.activation(warm[:], warm[:], F.Exp)

            # ---- Gram, per chunk ----
            if True:
                _p6cm = tc.tile_pool(name="p6", bufs=1)
                _p4cm = tc.tile_pool(name="p4", bufs=2)
                p6 = _p6cm.__enter__()
                p4 = _p4cm.__enter__()
                A6 = p6.tile([128, 6 * TC * 256], fp16, tag="A6", name="A6")
                B6 = p6.tile([128, 6 * TC * 128], fp16, tag="B6", name="B6")

                def reduce_groups(srcv, groups, tlo, tcn, width):
                    for (r0, r1, s0) in groups:
                        nc.vector.tensor_reduce(
                            Gst[:, s0 : s0 + (r1 - r0), tlo : tlo + tcn],
                            srcv[:, r0:r1, :, 0:width],
                            axis=AX.X,
                            op=OP.add,
                        )

                def tree(Abuf, Bbuf, npair, groups, tlo, tcn, eng_lvls=None):
                    # eng_lvls: widths handled by gpsimd instead of V
                    Av = Abuf[:].rearrange(
                        "p (r t c) -> p r t c", r=npair, t=TC
                    )[:, :, 0:tcn, :]
                    Bv = Bbuf[:].rearrange(
                        "p (r t c) -> p r t c", r=npair, t=TC
                    )[:, :, 0:tcn, :]
                    w = 256
                    src, dst = Av, Bv
                    while w > 8:
                        h = w // 2
                        eng = (
                            nc.gpsimd
                            if eng_lvls and w in eng_lvls
                            else nc.vector
                        )
                        eng.tensor_tensor(
                            out=dst[:, :, :, 0:h],
                            in0=src[:, :, :, 0:h],
                            in1=src[:, :, :, h:w],
                            op=OP.add,
                        )
                        src, dst = dst, src
                        w = h
                    return lambda: reduce_groups(src, groups, tlo, tcn, 8)

                OFF_GROUPS = [(0, 3, 1), (3, 5, 6), (5, 6, 11)]
                DIAG_GROUPS = [(0, 1, 0), (1, 2, 5), (2, 3, 10), (3, 4, 15)]
                GP_LVLS = {128, 64, 32, 16}   # diag L2..L5 on gpsimd

                pending = None
                for ci, (tlo, thi) in enumerate(CHUNKS):
                    tcn = thi - tlo
                    ts = slice(tlo, thi)
                    # interleave next sample's DMA issue into gpsimd stream
                    if tlo in (2, 8, 16):
                        issue_sample_dma({2: 1, 8: 2, 16: 3}[tlo])
                    A4 = p4.tile(
                        [128, 4 * TC * 256], fp16, tag="A4", name="A4"
                    )
                    B4 = p4.tile(
                        [128, 4 * TC * 128], fp16, tag="B4", name="B4"
                    )
                    A4v = A4[:].rearrange(
                        "p (r t c) -> p r t c", r=4, t=TC
                    )[:, :, 0:tcn, :]
                    for a in range(4):
                        nc.scalar.activation(
                            A4v[:, a, :, :], Xv[:, ts, a, :], F.Square
                        )
                    A6v = A6[:].rearrange(
                        "p (r t c) -> p r t c", r=6, t=TC
                    )[:, :, 0:tcn, :]
                    for r, (a, b) in enumerate(OFF_PAIRS):
                        nc.vector.tensor_tensor(
                            out=A6v[:, r, :, :],
                            in0=Xv[:, ts, a, :],
                            in1=Xv[:, ts, b, :],
                            op=OP.mult,
                        )
                    if pending is not None:
                        pending()  # deferred diag reduce of previous chunk
                    offred = tree(A6, B6, 6, OFF_GROUPS, tlo, tcn)
                    offred()
                    pending = tree(
                        A4, B4, 4, DIAG_GROUPS, tlo, tcn, eng_lvls=GP_LVLS
                    )
                pending()

            # ---- scalar algebra, batched across all tiles ----
            def t4(name):
                return sp.tile([128, NB * 4], fp32, tag=name, name=name)

            def t1(name):
                return sp.tile([128, NB], fp32, tag=name, name=name)

            def sqrt_ln_exp(dst, src, tmp):
                nc.scalar.activation(tmp[:], src[:], F.Ln)
                nc.scalar.activation(dst[:], tmp[:], F.Exp, scale=0.5)

            nc.vector.tensor_copy(Gv[:, :, 1:4, 0], Gv[:, :, 0, 1:4])
            nc.vector.tensor_copy(Gv[:, :, 2:4, 1], Gv[:, :, 1, 2:4])
            nc.vector.tensor_copy(Gv[:, :, 3, 2:3], Gv[:, :, 2, 3:4])

            d1 = t4("d1")
            d1v = d1[:].rearrange("p (t k) -> p t k", t=NB)
            nc.vector.tensor_reduce(d1v, Gv, axis=AX.X, op=OP.add)
            sig1 = t1("sig1")
            nc.vector.tensor_reduce(sig1[:], d1v, axis=AX.X, op=OP.add)
            sA = t1("sA")
            tmp1 = t1("tmp1")
            sqrt_ln_exp(sA, sig1, tmp1)
            den1 = t1("den1")
            nc.vector.tensor_scalar_add(den1[:], sA[:], 4.0 * (1.0 + EPS))
            r1 = t1("r1")
            nc.vector.reciprocal(r1[:], den1[:])
            score = t4("score")
            scv = score[:].rearrange("p (t k) -> p t k", t=NB)
            nc.vector.tensor_tensor(
                out=scv,
                in0=d1v,
                in1=r1[:].unsqueeze(2).broadcast_to([128, NB, 4]),
                op=OP.mult,
            )

            def softmax_weights(sc_v, ename):
                e = t4("e" + ename)
                ev = e[:].rearrange("p (t k) -> p t k", t=NB)
                sc_flat = sc_v.rearrange("p t k -> p (t k)")
                nc.scalar.activation(e[:], sc_flat, F.Exp, scale=0.5)
                se = t1("se" + ename)
                nc.vector.tensor_reduce(se[:], ev, axis=AX.X, op=OP.add)
                return ev, se

            e2v, se2 = softmax_weights(scv, "2")
            P4s = sp.tile([128, NB * 16], fp32, tag="P4s", name="P4s")
            P4v = P4s[:].rearrange("p (t a b) -> p t a b", t=NB, a=4)
            nc.vector.tensor_tensor(
                out=P4v,
                in0=Gv,
                in1=e2v.unsqueeze(2).broadcast_to([128, NB, 4, 4]),
                op=OP.mult,
            )
            u = t4("u")
            uv = u[:].rearrange("p (t k) -> p t k", t=NB)
            nc.vector.tensor_reduce(uv, P4v, axis=AX.X, op=OP.add)
            eu = t4("eu")
            euv = eu[:].rearrange("p (t k) -> p t k", t=NB)
            nc.vector.tensor_tensor(out=euv, in0=e2v, in1=uv, op=OP.mult)
            btb = t1("btb")
            nc.vector.tensor_reduce(btb[:], euv, axis=AX.X, op=OP.add)
            sB = t1("sB")
            tmp2 = t1("tmp2")
            sqrt_ln_exp(sB, btb, tmp2)
            DEN = t1("DEN")
            nc.vector.tensor_scalar(
                out=DEN[:],
                in0=se2[:],
                scalar1=4.0 * EPS,
                scalar2=1.0 + EPS,
                op0=OP.add,
                op1=OP.mult,
            )
            den2 = t1("den2")
            nc.vector.tensor_tensor(out=den2[:], in0=DEN[:], in1=sB[:], op=OP.add)
            r2 = t1("r2")
            nc.vector.reciprocal(r2[:], den2[:])
            s2t = t4("s2t")
            s2tv = s2t[:].rearrange("p (t k) -> p t k", t=NB)
            nc.vector.tensor_tensor(
                out=s2tv,
                in0=uv,
                in1=r2[:].unsqueeze(2).broadcast_to([128, NB, 4]),
                op=OP.mult,
            )
            score2 = t4("score2")
            sc2v = score2[:].rearrange("p (t k) -> p t k", t=NB)
            nc.vector.tensor_tensor(out=sc2v, in0=scv, in1=s2tv, op=OP.add)

            e3v, se3 = softmax_weights(sc2v, "3")
            den3 = t1("den3")
            nc.vector.tensor_scalar_add(den3[:], se3[:], 4.0 * EPS)
            q3 = t1("q3")
            nc.vector.reciprocal(q3[:], den3[:])
            W4 = sp.tile([128, NB * 4 * 4], fp16, tag="W4", name="W4")
            W4v = W4[:].rearrange("p (t k w) -> p t k w", t=NB, k=4)
            nc.vector.tensor_tensor(
                out=W4v[:, :, :, 0],
                in0=e3v,
                in1=q3[:].unsqueeze(2).broadcast_to([128, NB, 4]),
                op=OP.mult,
            )
            nc.vector.tensor_copy(W4v[:, :, :, 1:2], W4v[:, :, :, 0:1])
            nc.vector.tensor_copy(W4v[:, :, :, 2:4], W4v[:, :, :, 0:2])

            # ---- weighted output sum, per sample quarter ----
            if True:
                _wpcm = tc.tile_pool(name="wp", bufs=1)
                _wocm = tc.tile_pool(name="wo", bufs=2)
                wp = _wpcm.__enter__()
                wo = _wocm.__enter__()
                for q in range(4):
                    ts = slice(q * TC, (q + 1) * TC)
                    Pk = [
                        wp.tile([128, TC * C], fp16, tag=f"P{k}", name=f"P{k}")
                        for k in range(4)
                    ]
                    for k in range(4):
                        Pg = Pk[k][:].rearrange(
                            "p (t g w) -> p t g w", t=TC, w=4
                        )
                        Xg = Xv[:, ts, k, :].rearrange(
                            "p t (g w) -> p t g w", w=4
                        )
                        Wb = W4v[:, ts, k, :].unsqueeze(2).broadcast_to(
                            [128, TC, C // 4, 4]
                        )
                        nc.vector.tensor_tensor(
                            out=Pg, in0=Xg, in1=Wb, op=OP.mult
                        )
                    A01 = wp.tile([128, TC * C], fp16, tag="A01", name="A01")
                    A23 = wp.tile([128, TC * C], fp16, tag="A23", name="A23")
                    O = wo.tile([128, TC * C], fp16, tag="O", name="O")
                    nc.vector.tensor_tensor(
                        out=A01[:], in0=Pk[0][:], in1=Pk[1][:], op=OP.add
                    )
                    nc.vector.tensor_tensor(
                        out=A23[:], in0=Pk[2][:], in1=Pk[3][:], op=OP.add
                    )
                    nc.vector.tensor_tensor(
                        out=O[:], in0=A01[:], in1=A23[:], op=OP.add
                    )
                    Ov = O[:].rearrange("p (t c) -> p t c", t=TC)
                    dstv = out[q].rearrange("(ti ii) j c -> ii j ti c", ii=4)
                    for ii in range(4):
                        eng = nc.sync if (ii % 2 == 0) else nc.scalar
                        eng.dma_start(
                            out=dstv[ii],
                            in_=Ov[32 * ii : 32 * (ii + 1), :, :],
                        )

            _wocm.__exit__(None, None, None)
            _wpcm.__exit__(None, None, None)
            _p4cm.__exit__(None, None, None)
            _p6cm.__exit__(None, None, None)

    nc.compile()
    return nc


def _get_nc():
    if "nc" not in _cache:
        _cache["nc"] = _build_nc()
    return _cache["nc"]


def run_sharded(x, trace=False, **kw):
    from concourse.bass_utils import run_bass_kernel_spmd

    nc = _get_nc()
    x = np.ascontiguousarray(np.asarray(x), dtype=np.float32)
    in_maps = [{"x": x[i * B : (i + 1) * B]} for i in range(N_CORES)]
    res = run_bass_kernel_spmd(
        nc, in_maps, core_ids=list(range(N_CORES)), trace=trace, **kw
    )
    full = np.concatenate(
        [res.results[i]["out"] for i in range(N_CORES)], axis=0
    ).astype(np.float32)
    return full, res


def kernel(x):
    full, _ = run_sharded(x)
    return full
